# revision 20
# baseline (speedup 1.0000x reference)
"""Trainium2 Bass kernel for nn_AttentionHead (sparse/locally-connected attention).

Computation (per batch b):
    q = x @ (Wl*mask @ Wq*mask).T + (Wl*mask) @ bq        [S, H]
    k = x @ (Wk*mask).T + bk                              [S, H]
    v = x @ (Wv*mask).T + bv                              [S, H]
    scores = q @ k.T / sqrt(H)                            [S, S]
    probs  = softmax(scores, axis=-1)
    out    = probs @ v                                    [S, H]

Sharding: data-parallel over batch — core b computes batch b entirely
(weights replicated, no collectives).

The locality mask couples only units within Chebyshev distance 2 on a
32x32 grid (wrap-around), so at 128-row tile granularity (4 grid rows)
every masked weight matrix is block-tridiagonal (circulant): block
(I, J) is nonzero only for J in {I-1, I, I+1} mod 8. The folded
Wl*mask @ Wq*mask reaches +-4 grid rows = +-1 block, so it is block-
tridiagonal too. Projections therefore skip 5 of 8 contraction blocks.

On-core dataflow (all matmuls bf16 inputs, fp32 PSUM accumulate):
    xT   <- DMA-xbar-transpose(x)                 [h-part, s-free]
    qT,kT <- block-sparse W-stationary matmuls    [h'-part, s-free]
    v    <- xT-stationary block-sparse matmuls    [t-part, h-free]
    per 128-row block m:
        scores -> PSUM, ACT exp(+rowsum) -> E (bf16)
        probs  = E * (1/Z)  (DVE, per-partition scalar) -> DRAM
        ET     <- one DMA-xbar-transpose of E
        out    = (ET.T @ v) * (1/Z) -> DRAM
"""

import math

import ml_dtypes
import numpy as np

import concourse.bass as bass
import concourse.mybir as mybir
import concourse.tile as tile
from concourse import bacc, bass_utils

BF16 = ml_dtypes.bfloat16

B, S, H = 8, 2048, 1024
SQ = 5
P = 128
KT = H // P        # 8 feature tiles
ST = S // P        # 16 sequence blocks
NCH = S // 512     # 4 512-chunks over s/t
HCH = H // 512     # 2 512-chunks over h
N_CORES = 8

_cache = {}


def _locality_mask(hidden_size: int, width: int) -> np.ndarray:
    side = int(round(math.sqrt(hidden_size)))
    assert side * side == hidden_size
    r = np.arange(hidden_size) // side
    c = np.arange(hidden_size) % side
    dr = np.abs(r[:, None] - r[None, :])
    dc = np.abs(c[:, None] - c[None, :])
    dr = np.minimum(dr, side - dr)
    dc = np.minimum(dc, side - dc)
    half = width // 2
    return ((dr <= half) & (dc <= half)).astype(np.float32)


def _block_lists(support: np.ndarray):
    """support: [H, H] bool-ish. Returns blists[i] = sorted js with any
    nonzero in 128-block (i, j)."""
    blk = support.reshape(KT, P, KT, P).any(axis=(1, 3))
    return [sorted(np.nonzero(blk[i])[0].tolist()) for i in range(KT)]


def _mask_supports():
    mask = _locality_mask(H, SQ)
    sup1 = mask > 0                       # support of Wk', Wv' (symmetric)
    sup2 = (mask @ mask) > 0              # support of Wl'@Wq'
    return sup1, sup2


SQ_SCALE = 64.0   # host pre-scale on Wq_eff/bq_eff so fp8 q is well-ranged
SK_SCALE = 16.0   # host pre-scale on Wk/bk
NG = KT // 2      # 4 fp8 DoubleRow contraction groups of 256


def _build_program():
    f32 = mybir.dt.float32
    bf = mybir.dt.bfloat16
    f8 = mybir.dt.float8e4
    PSUM = bass.MemorySpace.PSUM
    Ident = mybir.ActivationFunctionType.Identity
    Exp = mybir.ActivationFunctionType.Exp
    DR = mybir.MatmulPerfMode.DoubleRow

    sup1, sup2 = _mask_supports()
    # For W.T block (k, m): nonzero iff W[m-block, k-block] nonzero.
    # sup is symmetric so row/col lists coincide; keep general anyway.
    nbr_kv = _block_lists(sup1)   # nbr_kv[k] = m/J blocks coupled to k
    nbr_q = _block_lists(sup2)
    NB1 = max(len(l) for l in nbr_kv)
    NBQ = max(len(l) for l in nbr_q)
    assert all(len(l) == NB1 for l in nbr_kv)
    assert all(len(l) == NBQ for l in nbr_q)

    nc = bacc.Bacc("TRN2", target_bir_lowering=False, debug=False)

    xt_d = nc.dram_tensor("xT", [H, S], bf, kind="ExternalInput")
    # packed nonzero 128x128 blocks of W.T, partition-major for one-shot DMA:
    # [p, k, b, c] = W.T[k*128+p, nbr[k][b]*128+c]
    wq_d = nc.dram_tensor("wqP", [P, KT, NBQ, P], bf, kind="ExternalInput")
    wk_d = nc.dram_tensor("wkP", [P, KT, NB1, P], bf, kind="ExternalInput")
    wv_d = nc.dram_tensor("wvP", [P, KT, NB1, P], bf, kind="ExternalInput")
    bq_d = nc.dram_tensor("bq", [P, KT], f32, kind="ExternalInput")
    bk_d = nc.dram_tensor("bk", [P, KT], f32, kind="ExternalInput")
    bv_d = nc.dram_tensor("bv", [H], f32, kind="ExternalInput")
    out_d = nc.dram_tensor("out", [S, H], f32, kind="ExternalOutput")
    probs_d = nc.dram_tensor("probs", [S, S], f32, kind="ExternalOutput")

    # m-order so each block's xT neighbors are among the earliest loads
    m_order = list(range(1, KT)) + [0]

    with tile.TileContext(nc) as tc:
        with (
            tc.tile_pool(name="sb", bufs=1) as sb,
            tc.tile_pool(name="work", bufs=2) as work,
            tc.tile_pool(name="stats", bufs=4) as stats,
            tc.tile_pool(name="ps", bufs=1, space=PSUM) as psp,
        ):
            # persistent activations: q/k in fp8 pair-layout for DoubleRow
            # scores (h' = 256*g + 128*i + p), v in bf16
            qT8 = [sb.tile([P, 2, S], f8, tag=f"qT{g}", name=f"qT{g}") for g in range(NG)]
            kT8 = [sb.tile([P, 2, S], f8, tag=f"kT{g}", name=f"kT{g}") for g in range(NG)]
            vt = [sb.tile([P, H], bf, tag=f"v{i}", name=f"v{i}") for i in range(ST)]
            # packed weights + biases + xT
            wq_all = sb.tile([P, KT, NBQ, P], bf, tag="wq", name="wq_all")
            wk_all = sb.tile([P, KT, NB1, P], bf, tag="wk", name="wk_all")
            wv_all = sb.tile([P, KT, NB1, P], bf, tag="wv", name="wv_all")
            wq_sb = [wq_all[:, k] for k in range(KT)]
            wk_sb = [wk_all[:, k] for k in range(KT)]
            wv_sb = [wv_all[:, k] for k in range(KT)]
            xT = [sb.tile([P, S], bf, tag=f"xT{k}", name=f"xT{k}") for k in range(KT)]
            bq_sb = sb.tile([P, KT], f32, tag="bq")
            bk_sb = sb.tile([P, KT], f32, tag="bk")
            bv_sb = sb.tile([P, H], f32, tag="bv")

            # dummy exp up front so the ACT table load happens before any
            # real dependency chain (it otherwise lands behind the input DMA
            # queue and stalls every downstream activation)
            dummy = stats.tile([P, 1], f32, tag="dmy", name="dummy")
            nc.vector.memset(dummy[:], 0.0)
            nc.scalar.activation(dummy[:], dummy[:], Exp)

            for k in range(KT):
                nc.sync.dma_start(out=xT[k][:], in_=xt_d.ap()[k * P:(k + 1) * P, :])
            nc.gpsimd.dma_start(out=wk_all[:], in_=wk_d.ap())
            nc.gpsimd.dma_start(out=wq_all[:], in_=wq_d.ap())
            nc.gpsimd.dma_start(out=wv_all[:], in_=wv_d.ap())
            nc.gpsimd.dma_start(out=bq_sb[:], in_=bq_d.ap())
            nc.gpsimd.dma_start(out=bk_sb[:], in_=bk_d.ap())
            bv_ap = bv_d.ap()
            bv_bcast = bass.AP(tensor=bv_ap.tensor, offset=bv_ap.offset,
                               ap=[[0, P]] + list(bv_ap.ap))
            nc.gpsimd.dma_start(out=bv_sb[:], in_=bv_bcast)

            # ---- kT, qT: block-sparse, weight-stationary, evac to fp8 ----
            for w_sb, b_sb, dstT, nbr in (
                (wk_sb, bk_sb, kT8, nbr_kv),
                (wq_sb, bq_sb, qT8, nbr_q),
            ):
                for m in m_order:
                    klist = [k for k in range(KT) if m in nbr[k]]
                    ps = [psp.tile([P, 512], f32, tag="ps", name="ps", bufs=5)
                          for _ in range(NCH)]
                    for ki, k in enumerate(klist):
                        bidx = nbr[k].index(m)
                        for j in range(NCH):
                            nc.tensor.matmul(
                                ps[j][:],
                                lhsT=w_sb[k][:, bidx, :],
                                rhs=xT[k][:, j * 512:(j + 1) * 512],
                                start=(ki == 0), stop=(ki == len(klist) - 1),
                            )
                    for j in range(NCH):
                        nc.scalar.activation(
                            dstT[m // 2][:, m % 2, j * 512:(j + 1) * 512],
                            ps[j][:], Ident, bias=b_sb[:, m:m + 1],
                        )

            # ---- v: xT-stationary, block-sparse over output chunks ----
            for i in range(ST):
                psv = [psp.tile([P, 512], f32, tag="ps", name="psv", bufs=5)
                       for _ in range(HCH)]
                for J in range(KT):
                    contribs = [kk for kk in range(KT) if J in nbr_kv[kk]]
                    for ci, k in enumerate(contribs):
                        bidx = nbr_kv[k].index(J)
                        nc.tensor.matmul(
                            psv[J // 4][:, (J % 4) * P:(J % 4 + 1) * P],
                            lhsT=xT[k][:, i * P:(i + 1) * P],
                            rhs=wv_sb[k][:, bidx, :],
                            start=(ci == 0), stop=(ci == len(contribs) - 1),
                        )
                for j in range(HCH):
                    nc.vector.tensor_add(
                        vt[i][:, j * 512:(j + 1) * 512], psv[j][:],
                        bv_sb[:, j * 512:(j + 1) * 512],
                    )

            # ---- attention ----
            inv_sqrt_h = float(1.0 / (math.sqrt(H) * SQ_SCALE * SK_SCALE))
            for m in range(ST):
                ms = slice(m * P, (m + 1) * P)
                E = work.tile([P, S], bf, tag="E", name="E")
                ET = work.tile([P, ST, P], bf, tag="ET", name="ET", bufs=3)
                zacc = stats.tile([P, NCH], f32, tag="zacc", name="zacc")
                for j in range(NCH):
                    js = slice(j * 512, (j + 1) * 512)
                    sc = psp.tile([P, 512], f32, tag="ps", name="sc", bufs=5)
                    for g in range(NG):
                        nc.tensor.matmul(
                            sc[:],
                            lhsT=qT8[g][:, :, ms],
                            rhs=kT8[g][:, :, js],
                            perf_mode=DR,
                            start=(g == 0), stop=(g == NG - 1),
                        )
                    nc.scalar.activation(
                        E[:, js], sc[:], Exp,
                        scale=inv_sqrt_h, accum_out=zacc[:, j:j + 1],
                    )
                    nc.sync.dma_start(out=ET[:, 4 * j:4 * (j + 1), :],
                                      in_=E[:, js], transpose=True)
                z = stats.tile([P, 1], f32, tag="z", name="z")
                nc.vector.reduce_sum(z[:], zacc[:], axis=mybir.AxisListType.X)
                r = stats.tile([P, 1], f32, tag="r", name="r")
                nc.vector.reciprocal(r[:], z[:])

                pr = work.tile([P, S], f32, tag="pr", name="pr")
                nc.vector.tensor_scalar_mul(pr[:], E[:], r[:])
                nc.gpsimd.dma_start(out=probs_d.ap()[ms, :], in_=pr[:])

                op = [psp.tile([P, 512], f32, tag="op", name="op", bufs=3)
                      for _ in range(HCH)]
                for k2 in range(ST):
                    for j in range(HCH):
                        nc.tensor.matmul(
                            op[j][:],
                            lhsT=ET[:, k2, :],
                            rhs=vt[k2][:, j * 512:(j + 1) * 512],
                            start=(k2 == 0), stop=(k2 == ST - 1),
                        )
                for j in range(HCH):
                    js = slice(j * 512, (j + 1) * 512)
                    ot = work.tile([P, 512], f32, tag="ot", name="ot")
                    nc.vector.tensor_scalar_mul(ot[:], op[j][:], r[:])
                    nc.gpsimd.dma_start(out=out_d.ap()[ms, js], in_=ot[:])

    nc.compile()
    return nc


def _prep_shared(Wq, bq, Wk, bk, Wv, bv, Wl):
    mask = _locality_mask(H, SQ)
    Wqm = Wq.astype(np.float32) * mask
    Wkm = Wk.astype(np.float32) * mask
    Wvm = Wv.astype(np.float32) * mask
    Wlm = Wl.astype(np.float32) * mask
    Wq_eff = (Wlm @ Wqm) * SQ_SCALE
    bq_eff = (Wlm @ bq.astype(np.float32)) * SQ_SCALE
    Wkm = Wkm * SK_SCALE
    bk = bk.astype(np.float32) * SK_SCALE

    sup1, sup2 = _mask_supports()
    nbr_kv = _block_lists(sup1)
    nbr_q = _block_lists(sup2)

    def pack(WT, nbr):
        # WT: [h, h'] = W.T. Partition-major pack of nonzero blocks:
        # out[p, k, b, :] = WT[k*128+p, nbr[k][b]*128 : +128].
        nb = len(nbr[0])
        outp = np.zeros((P, KT, nb, P), dtype=np.float32)
        for k in range(KT):
            for b, m in enumerate(nbr[k]):
                outp[:, k, b, :] = WT[k * P:(k + 1) * P, m * P:(m + 1) * P]
        return np.ascontiguousarray(outp).astype(BF16)

    return {
        "wqP": pack(np.ascontiguousarray(Wq_eff.T), nbr_q),
        "wkP": pack(np.ascontiguousarray(Wkm.T), nbr_kv),
        "wvP": pack(np.ascontiguousarray(Wvm.T), nbr_kv),
        "bq": np.ascontiguousarray(bq_eff.reshape(KT, P).T).astype(np.float32),
        "bk": np.ascontiguousarray(bk.astype(np.float32).reshape(KT, P).T),
        "bv": bv.astype(np.float32),
    }


def _make_in_maps(inputs):
    x = np.asarray(inputs["x"])
    shared = _prep_shared(
        np.asarray(inputs["Wq"]), np.asarray(inputs["bq"]),
        np.asarray(inputs["Wk"]), np.asarray(inputs["bk"]),
        np.asarray(inputs["Wv"]), np.asarray(inputs["bv"]),
        np.asarray(inputs["Wl"]),
    )
    in_maps = []
    for b in range(N_CORES):
        m = dict(shared)
        m["xT"] = np.ascontiguousarray(x[b].astype(BF16).T)
        in_maps.append(m)
    return in_maps


def _get_program():
    nc = _cache.get("nc")
    if nc is None:
        nc = _build_program()
        _cache["nc"] = nc
    return nc


def _run(inputs, trace=False, tmpdir=None):
    nc = _get_program()
    in_maps = _make_in_maps(inputs)
    res = bass_utils.run_bass_kernel_spmd(
        nc, in_maps, core_ids=list(range(N_CORES)), trace=trace, tmpdir=tmpdir,
    )
    out = np.stack([res.results[b]["out"] for b in range(N_CORES)])
    probs = np.stack([res.results[b]["probs"] for b in range(N_CORES)])
    return (out, probs), res


def kernel(**inputs):
    (out, probs), _ = _run(inputs)
    return out, probs


# revision 26
# speedup vs baseline: 1.2733x; 1.2733x over previous
"""Trainium2 Bass kernel for nn_AttentionHead (sparse/locally-connected attention).

Computation (per batch b):
    q = x @ (Wl*mask @ Wq*mask).T + (Wl*mask) @ bq        [S, H]
    k = x @ (Wk*mask).T + bk                              [S, H]
    v = x @ (Wv*mask).T + bv                              [S, H]
    scores = q @ k.T / sqrt(H)                            [S, S]
    probs  = softmax(scores, axis=-1)
    out    = probs @ v                                    [S, H]

Sharding: data-parallel over batch — core b computes batch b entirely
(weights replicated, no collectives).

The locality mask couples only units within Chebyshev distance 2 on a
32x32 grid (wrap-around), so at 128-row tile granularity (4 grid rows)
every masked weight matrix is block-tridiagonal (circulant): block
(I, J) is nonzero only for J in {I-1, I, I+1} mod 8. The folded
Wl*mask @ Wq*mask reaches +-4 grid rows = +-1 block, so it is block-
tridiagonal too. Projections therefore skip 5 of 8 contraction blocks.

On-core dataflow (all matmuls bf16 inputs, fp32 PSUM accumulate):
    xT   <- DMA-xbar-transpose(x)                 [h-part, s-free]
    qT,kT <- block-sparse W-stationary matmuls    [h'-part, s-free]
    v    <- xT-stationary block-sparse matmuls    [t-part, h-free]
    per 128-row block m:
        scores -> PSUM, ACT exp(+rowsum) -> E (bf16)
        probs  = E * (1/Z)  (DVE, per-partition scalar) -> DRAM
        ET     <- one DMA-xbar-transpose of E
        out    = (ET.T @ v) * (1/Z) -> DRAM
"""

import math

import ml_dtypes
import numpy as np

import concourse.bass as bass
import concourse.mybir as mybir
import concourse.tile as tile
from concourse import bacc, bass_utils
from concourse.masks import make_identity

BF16 = ml_dtypes.bfloat16

B, S, H = 8, 2048, 1024
SQ = 5
P = 128
KT = H // P        # 8 feature tiles
ST = S // P        # 16 sequence blocks
NCH = S // 512     # 4 512-chunks over s/t
HCH = H // 512     # 2 512-chunks over h
N_CORES = 8

_cache = {}


def _locality_mask(hidden_size: int, width: int) -> np.ndarray:
    side = int(round(math.sqrt(hidden_size)))
    assert side * side == hidden_size
    r = np.arange(hidden_size) // side
    c = np.arange(hidden_size) % side
    dr = np.abs(r[:, None] - r[None, :])
    dc = np.abs(c[:, None] - c[None, :])
    dr = np.minimum(dr, side - dr)
    dc = np.minimum(dc, side - dc)
    half = width // 2
    return ((dr <= half) & (dc <= half)).astype(np.float32)


def _block_lists(support: np.ndarray):
    """support: [H, H] bool-ish. Returns blists[i] = sorted js with any
    nonzero in 128-block (i, j)."""
    blk = support.reshape(KT, P, KT, P).any(axis=(1, 3))
    return [sorted(np.nonzero(blk[i])[0].tolist()) for i in range(KT)]


def _mask_supports():
    mask = _locality_mask(H, SQ)
    sup1 = mask > 0                       # support of Wk', Wv' (symmetric)
    sup2 = (mask @ mask) > 0              # support of Wl'@Wq'
    return sup1, sup2


SQ_SCALE = 64.0   # host pre-scale on Wq_eff/bq_eff so fp8 q is well-ranged
SK_SCALE = 16.0   # host pre-scale on Wk/bk
NG = KT // 2      # 4 fp8 DoubleRow contraction groups of 256


def _build_program():
    f32 = mybir.dt.float32
    bf = mybir.dt.bfloat16
    f8 = mybir.dt.float8e4
    PSUM = bass.MemorySpace.PSUM
    Ident = mybir.ActivationFunctionType.Identity
    Exp = mybir.ActivationFunctionType.Exp
    DR = mybir.MatmulPerfMode.DoubleRow

    sup1, sup2 = _mask_supports()
    # For W.T block (k, m): nonzero iff W[m-block, k-block] nonzero.
    # sup is symmetric so row/col lists coincide; keep general anyway.
    nbr_kv = _block_lists(sup1)   # nbr_kv[k] = m/J blocks coupled to k
    nbr_q = _block_lists(sup2)
    NB1 = max(len(l) for l in nbr_kv)
    NBQ = max(len(l) for l in nbr_q)
    assert all(len(l) == NB1 for l in nbr_kv)
    assert all(len(l) == NBQ for l in nbr_q)

    nc = bacc.Bacc("TRN2", target_bir_lowering=False, debug=False)

    xt_d = nc.dram_tensor("xT", [H, S], bf, kind="ExternalInput")
    # packed nonzero 128x128 blocks of W.T, partition-major for one-shot DMA:
    # [p, k, b, c] = W.T[k*128+p, nbr[k][b]*128+c]
    wq_d = nc.dram_tensor("wqP", [P, KT, NBQ, P], bf, kind="ExternalInput")
    wk_d = nc.dram_tensor("wkP", [P, KT, NB1, P], bf, kind="ExternalInput")
    wv_d = nc.dram_tensor("wvP", [P, KT, NB1, P], bf, kind="ExternalInput")
    bq_d = nc.dram_tensor("bq", [P, KT], f32, kind="ExternalInput")
    bk_d = nc.dram_tensor("bk", [P, KT], f32, kind="ExternalInput")
    bv_d = nc.dram_tensor("bv", [H], f32, kind="ExternalInput")
    out_d = nc.dram_tensor("out", [S, H], f32, kind="ExternalOutput")
    probs_d = nc.dram_tensor("probs", [S, S], f32, kind="ExternalOutput")

    # m-order so each block's xT neighbors are among the earliest loads
    m_order = list(range(1, KT)) + [0]

    with tile.TileContext(nc) as tc:
        with (
            tc.tile_pool(name="sb", bufs=1) as sb,
            tc.tile_pool(name="work", bufs=2) as work,
            tc.tile_pool(name="stats", bufs=4) as stats,
            tc.tile_pool(name="ps", bufs=1, space=PSUM) as psp,
        ):
            # persistent activations: q/k in fp8 pair-layout for DoubleRow
            # scores (h' = 256*g + 128*i + p), v in bf16
            qT8 = [sb.tile([P, 2, S], f8, tag=f"qT{g}", name=f"qT{g}") for g in range(NG)]
            kT8 = [sb.tile([P, 2, S], f8, tag=f"kT{g}", name=f"kT{g}") for g in range(NG)]
            vt = [sb.tile([P, H], bf, tag=f"v{i}", name=f"v{i}") for i in range(ST)]
            # packed weights + biases + xT
            wq_all = sb.tile([P, KT, NBQ, P], bf, tag="wq", name="wq_all")
            wk_all = sb.tile([P, KT, NB1, P], bf, tag="wk", name="wk_all")
            wv_all = sb.tile([P, KT, NB1, P], bf, tag="wv", name="wv_all")
            wq_sb = [wq_all[:, k] for k in range(KT)]
            wk_sb = [wk_all[:, k] for k in range(KT)]
            wv_sb = [wv_all[:, k] for k in range(KT)]
            xT = [sb.tile([P, S], bf, tag=f"xT{k}", name=f"xT{k}") for k in range(KT)]
            bq_sb = sb.tile([P, KT], f32, tag="bq")
            bk_sb = sb.tile([P, KT], f32, tag="bk")
            bv_sb = sb.tile([P, H], f32, tag="bv")

            # dummy exp up front so the ACT table load happens before any
            # real dependency chain (it otherwise lands behind the input DMA
            # queue and stalls every downstream activation)
            dummy = stats.tile([P, 1], f32, tag="dmy", name="dummy")
            nc.vector.memset(dummy[:], 0.0)
            nc.scalar.activation(dummy[:], dummy[:], Exp)

            ident = sb.tile([P, P], bf, tag="ident", name="ident")
            make_identity(nc, ident[:])

            for k in range(KT):
                nc.sync.dma_start(out=xT[k][:], in_=xt_d.ap()[k * P:(k + 1) * P, :])
            nc.gpsimd.dma_start(out=wk_all[:], in_=wk_d.ap())
            nc.gpsimd.dma_start(out=wq_all[:], in_=wq_d.ap())
            nc.gpsimd.dma_start(out=wv_all[:], in_=wv_d.ap())
            nc.gpsimd.dma_start(out=bq_sb[:], in_=bq_d.ap())
            nc.gpsimd.dma_start(out=bk_sb[:], in_=bk_d.ap())
            bv_ap = bv_d.ap()
            bv_bcast = bass.AP(tensor=bv_ap.tensor, offset=bv_ap.offset,
                               ap=[[0, P]] + list(bv_ap.ap))
            nc.gpsimd.dma_start(out=bv_sb[:], in_=bv_bcast)

            # ---- kT, qT: block-sparse, weight-stationary, evac to fp8 ----
            for w_sb, b_sb, dstT, nbr in (
                (wk_sb, bk_sb, kT8, nbr_kv),
                (wq_sb, bq_sb, qT8, nbr_q),
            ):
                for m in m_order:
                    klist = [k for k in range(KT) if m in nbr[k]]
                    ps = [psp.tile([P, 512], f32, tag="ps", name="ps", bufs=4)
                          for _ in range(NCH)]
                    for ki, k in enumerate(klist):
                        bidx = nbr[k].index(m)
                        for j in range(NCH):
                            nc.tensor.matmul(
                                ps[j][:],
                                lhsT=w_sb[k][:, bidx, :],
                                rhs=xT[k][:, j * 512:(j + 1) * 512],
                                start=(ki == 0), stop=(ki == len(klist) - 1),
                            )
                    for j in range(NCH):
                        nc.scalar.activation(
                            dstT[m // 2][:, m % 2, j * 512:(j + 1) * 512],
                            ps[j][:], Ident, bias=b_sb[:, m:m + 1],
                        )

            # ---- v: xT-stationary, block-sparse over output chunks ----
            for i in range(ST):
                psv = [psp.tile([P, 512], f32, tag="ps", name="psv", bufs=4)
                       for _ in range(HCH)]
                for J in range(KT):
                    contribs = [kk for kk in range(KT) if J in nbr_kv[kk]]
                    for ci, k in enumerate(contribs):
                        bidx = nbr_kv[k].index(J)
                        nc.tensor.matmul(
                            psv[J // 4][:, (J % 4) * P:(J % 4 + 1) * P],
                            lhsT=xT[k][:, i * P:(i + 1) * P],
                            rhs=wv_sb[k][:, bidx, :],
                            start=(ci == 0), stop=(ci == len(contribs) - 1),
                        )
                for j in range(HCH):
                    nc.vector.tensor_add(
                        vt[i][:, j * 512:(j + 1) * 512], psv[j][:],
                        bv_sb[:, j * 512:(j + 1) * 512],
                    )

            # ---- attention ----
            inv_sqrt_h = float(1.0 / (math.sqrt(H) * SQ_SCALE * SK_SCALE))
            for m in range(ST):
                ms = slice(m * P, (m + 1) * P)
                E = work.tile([P, S], bf, tag="E", name="E")
                ET = work.tile([P, ST, P], bf, tag="ET", name="ET", bufs=3)
                zacc = stats.tile([P, NCH], f32, tag="zacc", name="zacc")
                tp = [psp.tile([P, ST // 2, P], bf, tag="tp", name="tp", bufs=2)
                      for _ in range(2)]
                for j in range(NCH):
                    js = slice(j * 512, (j + 1) * 512)
                    sc = psp.tile([P, 512], f32, tag="ps", name="sc", bufs=4)
                    for g in range(NG):
                        nc.tensor.matmul(
                            sc[:],
                            lhsT=qT8[g][:, :, ms],
                            rhs=kT8[g][:, :, js],
                            perf_mode=DR,
                            start=(g == 0), stop=(g == NG - 1),
                        )
                    nc.scalar.activation(
                        E[:, js], sc[:], Exp,
                        scale=inv_sqrt_h, accum_out=zacc[:, j:j + 1],
                    )
                    # transpose the four 128x128 sub-blocks of this chunk on
                    # PE (one accumulation group per PSUM bank-tile, disjoint
                    # slices)
                    for jj in range(4 * j, 4 * j + 4):
                        nc.tensor.matmul(
                            tp[jj // 8][:, jj % 8, :],
                            lhsT=E[:, jj * P:(jj + 1) * P],
                            rhs=ident[:],
                            is_transpose=True,
                            start=(jj % 8 == 0), stop=(jj % 8 == 7),
                        )
                    if j % 2 == 1:
                        h = j // 2
                        nc.vector.tensor_copy(
                            ET[:, h * 8:(h + 1) * 8, :], tp[h][:])
                z = stats.tile([P, 1], f32, tag="z", name="z")
                nc.vector.reduce_sum(z[:], zacc[:], axis=mybir.AxisListType.X)
                r = stats.tile([P, 1], f32, tag="r", name="r")
                nc.vector.reciprocal(r[:], z[:])

                pr = work.tile([P, S], f32, tag="pr", name="pr")
                nc.vector.tensor_scalar_mul(pr[:], E[:], r[:])
                nc.sync.dma_start(out=probs_d.ap()[ms, :], in_=pr[:])

                op = [psp.tile([P, 512], f32, tag="op", name="op", bufs=2)
                      for _ in range(HCH)]
                for k2 in range(ST):
                    for j in range(HCH):
                        nc.tensor.matmul(
                            op[j][:],
                            lhsT=ET[:, k2, :],
                            rhs=vt[k2][:, j * 512:(j + 1) * 512],
                            start=(k2 == 0), stop=(k2 == ST - 1),
                        )
                for j in range(HCH):
                    js = slice(j * 512, (j + 1) * 512)
                    ot = work.tile([P, 512], f32, tag="ot", name="ot")
                    nc.vector.tensor_scalar_mul(ot[:], op[j][:], r[:])
                    nc.sync.dma_start(out=out_d.ap()[ms, js], in_=ot[:])

    nc.compile()
    return nc


def _prep_shared(Wq, bq, Wk, bk, Wv, bv, Wl):
    mask = _locality_mask(H, SQ)
    Wqm = Wq.astype(np.float32) * mask
    Wkm = Wk.astype(np.float32) * mask
    Wvm = Wv.astype(np.float32) * mask
    Wlm = Wl.astype(np.float32) * mask
    Wq_eff = (Wlm @ Wqm) * SQ_SCALE
    bq_eff = (Wlm @ bq.astype(np.float32)) * SQ_SCALE
    Wkm = Wkm * SK_SCALE
    bk = bk.astype(np.float32) * SK_SCALE

    sup1, sup2 = _mask_supports()
    nbr_kv = _block_lists(sup1)
    nbr_q = _block_lists(sup2)

    def pack(WT, nbr):
        # WT: [h, h'] = W.T. Partition-major pack of nonzero blocks:
        # out[p, k, b, :] = WT[k*128+p, nbr[k][b]*128 : +128].
        nb = len(nbr[0])
        outp = np.zeros((P, KT, nb, P), dtype=np.float32)
        for k in range(KT):
            for b, m in enumerate(nbr[k]):
                outp[:, k, b, :] = WT[k * P:(k + 1) * P, m * P:(m + 1) * P]
        return np.ascontiguousarray(outp).astype(BF16)

    return {
        "wqP": pack(np.ascontiguousarray(Wq_eff.T), nbr_q),
        "wkP": pack(np.ascontiguousarray(Wkm.T), nbr_kv),
        "wvP": pack(np.ascontiguousarray(Wvm.T), nbr_kv),
        "bq": np.ascontiguousarray(bq_eff.reshape(KT, P).T).astype(np.float32),
        "bk": np.ascontiguousarray(bk.astype(np.float32).reshape(KT, P).T),
        "bv": bv.astype(np.float32),
    }


def _make_in_maps(inputs):
    x = np.asarray(inputs["x"])
    shared = _prep_shared(
        np.asarray(inputs["Wq"]), np.asarray(inputs["bq"]),
        np.asarray(inputs["Wk"]), np.asarray(inputs["bk"]),
        np.asarray(inputs["Wv"]), np.asarray(inputs["bv"]),
        np.asarray(inputs["Wl"]),
    )
    in_maps = []
    for b in range(N_CORES):
        m = dict(shared)
        m["xT"] = np.ascontiguousarray(x[b].astype(BF16).T)
        in_maps.append(m)
    return in_maps


def _get_program():
    nc = _cache.get("nc")
    if nc is None:
        nc = _build_program()
        _cache["nc"] = nc
    return nc


def _run(inputs, trace=False, tmpdir=None):
    nc = _get_program()
    in_maps = _make_in_maps(inputs)
    res = bass_utils.run_bass_kernel_spmd(
        nc, in_maps, core_ids=list(range(N_CORES)), trace=trace, tmpdir=tmpdir,
    )
    out = np.stack([res.results[b]["out"] for b in range(N_CORES)])
    probs = np.stack([res.results[b]["probs"] for b in range(N_CORES)])
    return (out, probs), res


def kernel(**inputs):
    (out, probs), _ = _run(inputs)
    return out, probs
